# revision 25
# baseline (speedup 1.0000x reference)
"""Trainium2 Bass kernel for nn_EquivariantCrossAttention.

Sharding: batch*query rows (2*256=512) split across 8 cores (64 queries each,
cores 0-3 -> batch 0, cores 4-7 -> batch 1). k/v/a replicated per batch.

Per-core layout: feature-on-partition, (c,z) flattened on the free dim.
64 queries x 128 latents = 8192 free columns, processed in 16 chunks of 512.

Key implementation notes (validated vs reference in a numpy quantization
model and on hardware):
  - The PE's float32r mode is ~10-bit-mantissa round-to-nearest (tf32-like).
    A 4-byte LDWEIGHTS costs ~185 ns and cannot hide behind the previous
    matmul, so weight-switching fp32r matmuls run at 327-427 ns vs 234 ns
    for weight-reusing ones.  bf16 weights+operands stream at ~240-265 ns.
    Walrus forbids mixing 32-bit and 16-bit matmul operands, so chains flip
    to bf16 wholesale.  Per the numpy error model, bf16 is near-free for
    the B1 q/k/logits chain, the mW1/G chain, and masks/selectors, but the
    mW2 output chain (no LN downstream) and the pg/Wbm/h1r pair cost real
    accuracy - those stay fp32r, as do the RFF phase path, logits + softmax
    (fp32), LN statistics, and the output projection.
  - Phase order is B0 -> PRE -> B1: engine queues are strict FIFO, so PRE's
    ACT ops (waiting on the late weight DMA) must not queue ahead of B0's
    sin chain.  Constants arrive in four DMAs ordered by first use.
  - HAM warm filler (junk matmuls) is load-bearing: without it the PE clock
    gate drops to 4/8 in ACT/DVE-bound stretches and can stay cold for tens
    of microseconds even under dense load afterwards.  Junk pools use
    bufs=2 so filler pipelines at ~235 ns instead of serializing at ~770.
  - RFF dense biases folded into downstream weights (bqe->bq', bve->vb1').
  - LayerNorm gain/bias folded into the following matmul (W'=g*W, b'=bn@W+b).
  - vLN mean removed with one subtract; vLN rstd multiplied into h1 once
    (h1r) and commuted through the mW1/Wbm matmuls.
  - mixer-LN mean via rank-1 matmul fold (csmW2 x -mean); mixer rstd folded
    into the 8-row attention tile (attR) instead of the 128-row v2.
  - rstd = exp(-0.5*ln(var+eps)) so LN and softmax share the natural_log_exp
    activation-table set (hardware Rsqrt is forbidden; table swaps cost 2.7us).
  - FiLM: G = va*(pg + bgam1), the bias applied as a per-partition ACT bias
    while evacuating pg from PSUM (cheaper than the old amwT/eyeZ matmul).
  - LN statistics accumulated straight into multi-partition PSUM rows via
    one-hot selector matmuls (selS bf16 / selF fp32), read by ln_math.
  - v3 bias folded into the output projection bias via softmax-sum=1.
  - Softmax without max subtraction; exp+normalize per quarter, in place on
    log_all, inside the Ln/Exp window.  The exp is pinned into its quarter
    via a zero bias column derived from h2_q; otherwise the scheduler hoists
    it into B1, paying two gelu<->ln_exp ACT-table reloads there per quarter.
"""
import sys
import numpy as np

for _p in ("/opt/trn_rl_repo",):
    if _p not in sys.path:
        sys.path.insert(0, _p)

import ml_dtypes
import concourse.bass as bass
import concourse.tile as tile
from concourse import bacc, mybir
from concourse.bass_utils import run_bass_kernel_spmd

FP = mybir.dt.float32
FR = mybir.dt.float32r
BF = mybir.dt.bfloat16
AF = mybir.ActivationFunctionType
OP = mybir.AluOpType
AX = mybir.AxisListType
ts = bass.ts

GELU_AF = AF.Gelu_apprx_tanh  # sim_test overrides (sim lacks gelu)

B, C, Z, D = 2, 256, 128, 3
H, NH, HH = 64, 8, 512
EPS = 1e-5
NCORE = 8
CPC = (B * C) // NCORE          # 64 queries per core
QC = 4                          # queries per chunk
CZ = QC * Z                     # 512 free columns per chunk
NCHUNK = CPC // QC              # 16
QSPLIT = 4                      # process h2 in quarters (SBUF)
CPQ = NCHUNK // QSPLIT          # 4 chunks per quarter
CZALL = CPC * Z                 # 8192


def _fp(ap):
    """Read a float32r AP as plain fp32 (same bits) for DVE/ACT consumers."""
    return ap.bitcast(FP)


# packed-constant layouts: (name, base_row, nrows, ncols)
# cpk_a (fp32, DMA 1): everything B0 needs + ACT columns + fp32 selectors
CPA_LAYOUT = [
    ("xp", 0, D, CPC + Z), ("Bcat", 0, D, 128), ("qb", 0, 1, 128),
    ("Wqv", 0, 128, 128), ("bqc", 0, 128, 4), ("vb1p", 0, H, 1),
    ("bcol", 0, 128, 8), ("onec", 0, 128, 1), ("oner", 0, 1, CZ),
    ("selF", 0, 128, NCHUNK * NCHUNK),
]
# cpk_b (fp32, DMA 2): PRE k/v chain + the accuracy-critical B2 weights
CPB_LAYOUT = [
    ("aT", 0, H, Z), ("WkF", 0, H, HH), ("WvF", 0, H, HH),
    ("bkv", 0, 1, 2 * HH), ("WgamF", 0, H, HH), ("WbmF", 0, H, HH),
]
# wpk_w (bf16, DMA 3): B1 q chain + masks + mW1 chain
WPW_LAYOUT = [
    ("Wq", 0, H, HH), ("vW1b", 64, H, H), ("maskS", 0, 128, 32),
    ("selS", 0, 128, NCHUNK * NCHUNK), ("eyeZ", 0, 128, CZ),
    ("maskB", 64, NH, HH), ("mW1", 0, 128, 4 * HH), ("csmW2b", 0, 1, HH),
]
# cpk_d (fp32, DMA 4): mW2 chain + output projection (quarters onward)
CPD_LAYOUT = [
    ("mW2F", 0, 128, 4 * HH), ("maskT", 64, NH, 4 * 128),
    ("WoF", 0, 128, 4 * HH), ("boppF", 0, 1, HH),
]


def _cols(layout):
    d = {}
    c = 0
    for n, r, nr, ncol in layout:
        d[n] = (r, nr, c, ncol)
        c += ncol
    return d, c


CPA_COLS, CPA_NCOL = _cols(CPA_LAYOUT)
CPB_COLS, CPB_NCOL = _cols(CPB_LAYOUT)
WPW_COLS, WPW_NCOL = _cols(WPW_LAYOUT)
CPD_COLS, CPD_NCOL = _cols(CPD_LAYOUT)


def _bc(ap, outer):
    """[P,n] -> [P,outer,n] with stride-0 outer dim (broadcast over queries)."""
    return bass.AP(tensor=ap.tensor, offset=ap.offset,
                   ap=[ap.ap[0], [0, outer]] + list(ap.ap[1:]))


def _pbc(ap, nparts):
    """[1,n] -> [nparts,n] partition-broadcast AP (stride-0 partitions; DMA only)."""
    return bass.AP(tensor=ap.tensor, offset=ap.offset,
                   ap=[[0, nparts]] + list(ap.ap[1:]))


def _bc_inner(ap, inner):
    """[P,n] -> [P,n,inner] with stride-0 inner dim."""
    return bass.AP(tensor=ap.tensor, offset=ap.offset,
                   ap=list(ap.ap) + [[0, inner]])


def build_kernel():
    nc = bacc.Bacc("TRN2", target_bir_lowering=False, debug=False,
                   num_devices=NCORE)

    t = {}
    t["cpk_a"] = nc.dram_tensor("cpk_a", [128, CPA_NCOL], FR,
                                kind="ExternalInput").ap()
    t["cpk_b"] = nc.dram_tensor("cpk_b", [128, CPB_NCOL], FR,
                                kind="ExternalInput").ap()
    t["wpk_w"] = nc.dram_tensor("wpk_w", [128, WPW_NCOL], BF,
                                kind="ExternalInput").ap()
    t["cpk_d"] = nc.dram_tensor("cpk_d", [128, CPD_NCOL], FR,
                                kind="ExternalInput").ap()
    t["out"] = nc.dram_tensor("out", [CPC, HH], FP, kind="ExternalOutput").ap()

    with tile.TileContext(nc) as tc:
        body(tc, t)
    nc.finalize()
    return nc


def body(tc, t):
    nc = tc.nc
    t = dict(t)
    t["scr_mr"] = nc.dram_tensor("scr_mr", [NCHUNK, 2, CZ], BF,
                                 kind="Internal").ap()
    t["scr_rm"] = nc.dram_tensor("scr_rm", [NCHUNK, CZ], BF, kind="Internal").ap()
    import contextlib
    stack = contextlib.ExitStack()
    P_const = stack.enter_context(tc.tile_pool(name="const", bufs=1))
    P_big = stack.enter_context(tc.tile_pool(name="big", bufs=1))

    cpa = P_const.tile([128, CPA_NCOL], FR, tag="cpa")
    cpb = P_const.tile([128, CPB_NCOL], FR, tag="cpb")
    wpw = P_const.tile([128, WPW_NCOL], BF, tag="wpw")
    cpd = P_const.tile([128, CPD_NCOL], FR, tag="cpd")
    # ordered by first use: B0 needs only cpa; PRE cpb; B1 wpw; quarters cpd
    nc.sync.dma_start(cpa[:], t["cpk_a"])
    nc.sync.dma_start(cpb[:], t["cpk_b"])
    nc.sync.dma_start(wpw[:], t["wpk_w"])
    nc.sync.dma_start(cpd[:], t["cpk_d"])

    S = {}
    for tl, cols in ((cpa, CPA_COLS), (cpb, CPB_COLS), (wpw, WPW_COLS),
                     (cpd, CPD_COLS)):
        for n, (r0, nr, c0, ncol) in cols.items():
            S[n] = tl[r0:r0 + nr, c0:c0 + ncol]
    S["xT"] = S["xp"][:, 0:CPC]
    S["pT"] = S["xp"][:, CPC:CPC + Z]
    S["bgam1"], S["mb1pp"] = S["bcol"][:, 0:4], S["bcol"][:, 4:8]
    maskB_hi = S["maskB"]
    mW1_s = S["mW1"].rearrange("p (j n) -> p j n", j=4)
    mW2_s = S["mW2F"].rearrange("p (j n) -> p j n", j=4)
    Wo_s = S["WoF"].rearrange("p (j n) -> p j n", j=4)
    ones_r = S["oner"]          # [1,CZ]  float32r ones

    eps_c = P_const.tile([128, 1], FP)
    nc.vector.memset(eps_c[:], EPS)
    wj = P_const.tile([128, CZ], BF)
    nc.vector.memset(wj[:], 0.0)
    wjf = P_const.tile([128, CZ], FP)
    nc.vector.memset(wjf[:], 0.0)

    def _warm(pool, n, ncols=CZ):
        """Throwaway bf16 matmuls (~235 ns at N=512 warm): keep the PE HAM
        clock-gate at 8/8 wherever real matmul density dips.  Load-bearing:
        once the gate drops to 4/8 it can stay cold for tens of us even
        under a saturated half-clocked matmul stream."""
        for _r in range(n):
            jp = pool.tile([128, CZ], FP, tag="junk")
            nc.tensor.matmul(jp[:, 0:ncols], wj[:, 0:128], wj[:, 0:ncols],
                             start=True, stop=True)

    def _warm_f(pool, n):
        """fp32 filler (~1.2 us each): for multi-us PE-idle windows."""
        for _r in range(n):
            jp = pool.tile([128, CZ], FP, tag="junk")
            nc.tensor.matmul(jp[:], wjf[:, 0:128], wjf[:],
                             start=True, stop=True)

    # dense burst while the cpk_a DMA streams in
    with tc.tile_pool(name="warm_ps", bufs=2, space="PSUM") as WP:
        _warm(WP, 16)

    # persistent buffers
    h1_all = P_big.tile([H, CZALL], BF)         # B1 gelu output
    log_all = P_big.tile([64 + NH, CZALL], FP)  # rows 64-71: logits -> att
    y_all = P_big.tile([128, 4, CPC], FR)
    MvRv = P_big.tile([NCHUNK, 2, CZ], BF)  # vLN mean | rstd, interleaved
    nMq = P_big.tile([CPQ, CZ], BF)
    RmQ = P_big.tile([CPQ, CZ], BF)
    nMm1 = P_big.tile([1, CPQ * CZ], BF)   # one quarter's negated means, row form
    esum_all = P_big.tile([64 + NH, CPC], FR)  # softmax 1/sum, rows 64-71
    kv_s = P_big.tile([128, 4, Z], FP)
    va_s = P_big.tile([128, 4, Z], FP)
    bqkT_s = P_big.tile([Z, NH], BF)       # [z, h] = 0.125 * sum_{f in h} bq[f]k[f,z]

    # ---------------- B0: inv -> RFF -> ie (sin); needs only cpk_a --------
    with tc.tile_pool(name="iep", bufs=1) as ie_pool:
        ie_all = ie_pool.tile([128, CZALL], BF)
        with tc.tile_pool(name="b0_ps", bufs=2, space="PSUM") as PP, \
             tc.tile_pool(name="b0_jk", bufs=2, space="PSUM") as JP0, \
             tc.tile_pool(name="pre_ps", bufs=1, space="PSUM") as PPP, \
             tc.tile_pool(name="pre_sb", bufs=1) as PSB, \
             tc.tile_pool(name="b0_sb", bufs=3) as SB:
            RC = 12582912.0  # 1.5 * 2^23: fp32 add rounds to nearest integer
            for i in range(NCHUNK):
                _warm(JP0, 4)
                cols = ts(i, CZ)
                inv = SB.tile([D, QC, Z], FR, tag="inv")
                nc.vector.tensor_sub(
                    inv[:], _bc_inner(_fp(S["xT"])[:, ts(i, QC)], Z),
                    _bc(_fp(S["pT"])[:, :], QC))
                # rows: [m_q, m_q+0.25, m_v, m_v+0.25] (unit-period phases)
                mm = PP.tile([128, CZ], FP, tag="mm")
                nc.tensor.matmul(mm[:], S["Bcat"][:], inv[:], start=True,
                                 stop=False)
                nc.tensor.matmul(mm[:], S["qb"][:], ones_r[:], start=False,
                                 stop=True)
                r1 = SB.tile([128, CZ], FP, tag="r1")
                nc.scalar.activation(r1[:], mm[:], AF.Copy, bias=RC)
                fr = SB.tile([128, CZ], FP, tag="fr")
                nc.vector.scalar_tensor_tensor(fr[:], r1[:], RC, mm[:],
                                               op0=OP.subtract,
                                               op1=OP.subtract)
                F = SB.tile([128, CZ], FR, tag="F")
                nc.scalar.activation(F[:], fr[:], AF.Sin,
                                     scale=float(2 * np.pi))
                ieps = PP.tile([128, CZ], FP, tag="ieps")
                nc.tensor.matmul(ieps[:], S["Wqv"][:], F[:],
                                 start=True, stop=True)
                nc.scalar.copy(ie_all[:, cols], ieps[:])

            # ---- PRE: k, va, bqkT (needs cpk_b; overlaps B0 on the PE) ----
            bkv = S["bkv"].rearrange("p (k n) -> p k n", k=2)
            # kv_s carries the 1/sqrt(H)=0.125 attention scale
            for dst_s, W_n, bi, scl in [(kv_s, "WkF", 0, 0.125),
                                        (va_s, "WvF", 1, 1.0)]:
                for tt in range(4):
                    ps = PPP.tile([128, Z], FP, tag="kv")
                    nc.tensor.matmul(ps[:], S[W_n][:, ts(tt, 128)],
                                     S["aT"][:], start=True, stop=False)
                    nc.tensor.matmul(ps[:], bkv[:, bi, ts(tt, 128)],
                                     ones_r[:, 0:Z], start=False, stop=True)
                    nc.scalar.activation(dst_s[:, tt, :], ps[:], AF.Copy,
                                         scale=scl)
            # bqkT[z, h] = 0.125 * sum_f bq[f] k[f,z] [head(f)==h]
            bqk_ps = PPP.tile([Z, NH], FP, tag="bqk")
            for tt in range(4):
                ek0 = PSB.tile([128, Z], BF, tag="ek0")
                nc.scalar.mul(ek0[:], kv_s[:, tt, :],
                              _fp(S["bqc"][:, tt:tt + 1]))
                nc.tensor.matmul(bqk_ps[:], ek0[:], S["maskS"][:, ts(tt, NH)],
                                 start=(tt == 0), stop=(tt == 3))
            nc.scalar.copy(bqkT_s[:], bqk_ps[:])

        # ---- B1: q/logits, h1, vLN stats into PSUM (gelu) ----
        with tc.tile_pool(name="b1_st", bufs=1, space="PSUM") as PPS1:
            SvP = PPS1.tile([NCHUNK, CZ], FP)
            QvP = PPS1.tile([NCHUNK, CZ], FP)
            with tc.tile_pool(name="b1_ps", bufs=1, space="PSUM") as PP, \
                 tc.tile_pool(name="b1_qps", bufs=3, space="PSUM") as PPQ, \
                 tc.tile_pool(name="b1_jk", bufs=1, space="PSUM") as JP1, \
                 tc.tile_pool(name="b1_ek", bufs=8) as SBE, \
                 tc.tile_pool(name="b1_sb", bufs=2) as SB:
                for i in range(NCHUNK):
                    _warm(JP1, 1)
                    cols = ts(i, CZ)
                    # h1 path first: its gelu/square run on ACT while the PE
                    # works through the q-path matmuls below
                    h1ps = PP.tile([H, CZ], FP, tag="h1ps")
                    nc.tensor.matmul(h1ps[:], S["vW1b"][:],
                                     ie_all[64:128, cols],
                                     start=True, stop=True)
                    qpss = []
                    for tt in range(4):
                        qps = PPQ.tile([128, CZ], FP, tag="qps")
                        nc.tensor.matmul(qps[:], S["Wq"][:, ts(tt, 128)],
                                         ie_all[0:64, cols],
                                         start=True, stop=True)
                        qpss.append(qps)
                    nc.scalar.activation(h1_all[:, cols], h1ps[:], GELU_AF,
                                         bias=_fp(S["vb1p"])[:])
                    sq = SB.tile([H, CZ], BF, tag="sq")
                    nc.scalar.square(sq[:], h1_all[:, cols])
                    eks = []
                    for tt in range(4):
                        ek = SBE.tile([128, CZ], BF, tag="ek")
                        nc.vector.tensor_mul(ek[:], qpss[tt][:],
                                             _bc(kv_s[:, tt, :], QC))
                        eks.append(ek)
                    lps = PP.tile([NH, CZ], FP, tag="lps")
                    for tt in range(4):
                        nc.tensor.matmul(lps[:], S["maskS"][:, ts(tt, NH)],
                                         eks[tt][:], start=(tt == 0),
                                         stop=False)
                    nc.tensor.matmul(lps[:], bqkT_s[:], S["eyeZ"][:],
                                     start=False, stop=True)
                    nc.scalar.copy(log_all[64:64 + NH, cols], lps[:])
                    sel = S["selS"][0:64, ts(i, NCHUNK)]
                    nc.tensor.matmul(SvP[:], sel, h1_all[:, cols],
                                     start=(i == 0), stop=(i == NCHUNK - 1))
                    nc.tensor.matmul(QvP[:], sel, sq[:],
                                     start=(i == 0), stop=(i == NCHUNK - 1))

            # ---- C1: vLN rstd (ln/exp); stats read from PSUM in place ----
            ln_math(nc, slice(0, NCHUNK), SvP, QvP, MvRv[:, 0, :], float(H),
                    False, MvRv[:, 1, :], eps_c)
            nc.sync.dma_start(t["scr_mr"], MvRv[:])
            with tc.tile_pool(name="c1_jk", bufs=2, space="PSUM") as JPC:
                _warm_f(JPC, 3)

    # ---------------- quarters: B2 (gelu) -> ln+softmax -> D -------------
    h2_pool = stack.enter_context(tc.tile_pool(name="h2p", bufs=1))
    h2_q = h2_pool.tile([128, 4, CPQ * CZ], FR)
    P_bc = stack.enter_context(tc.tile_pool(name="bcast", bufs=8))

    def prefetch_mr(qq):
        """Broadcast each chunk's vLN (mean|rstd) row pair to H rows."""
        pf = []
        for ii in range(CPQ):
            i = qq * CPQ + ii
            mr = P_bc.tile([H, 2, CZ], BF, tag="mr")
            src = t["scr_mr"][i, :, :]
            nc.sync.dma_start(mr[:], bass.AP(tensor=src.tensor,
                                             offset=src.offset,
                                             ap=[[0, H]] + list(src.ap)))
            pf.append(mr)
        return pf

    pf_cur = prefetch_mr(0)
    for qq in range(QSPLIT):
        with tc.tile_pool(name="b2_st", bufs=1, space="PSUM") as PPS:
            SmP = PPS.tile([CPQ, CZ], FP, tag="SmP")
            QmP = PPS.tile([CPQ, CZ], FP, tag="QmP")
            with tc.tile_pool(name="qb_jk", bufs=2, space="PSUM") as JPQ:
                _warm(JPQ, 6)
            with tc.tile_pool(name="b2_pg", bufs=2, space="PSUM") as PPG, \
                 tc.tile_pool(name="b2_v1", bufs=4, space="PSUM") as PPV, \
                 tc.tile_pool(name="b2_s4", bufs=4) as SB4, \
                 tc.tile_pool(name="b2_sb", bufs=3) as SB, \
                 tc.tile_pool(name="b2_g4", bufs=16) as SBG:
                h1rs = []
                for ii in range(CPQ):
                    cols = ts(qq * CPQ + ii, CZ)
                    h1c = SB.tile([H, CZ], FP, tag="h1c")
                    nc.vector.tensor_sub(h1c[:], h1_all[:, cols],
                                         pf_cur[ii][:, 0, :])
                    h1r = SB4.tile([H, CZ], FR, tag="h1r")
                    nc.vector.tensor_mul(h1r[:], h1c[:], pf_cur[ii][:, 1, :])
                    h1rs.append(h1r)
                # pg/G for the whole quarter, tt-major: each WgamF slice's
                # 4-byte LDWEIGHTS is amortized over the 4 chunks
                Gs = {}
                for tt in range(4):
                    for ii in range(CPQ):
                        pg = PPG.tile([128, CZ], FP, tag="pg")
                        nc.tensor.matmul(pg[:], S["WgamF"][:, ts(tt, 128)],
                                         h1rs[ii][:], start=True, stop=True)
                        # FiLM bias folded in while evacuating PSUM
                        pgb = SB.tile([128, CZ], FP, tag="pgb")
                        nc.scalar.activation(pgb[:], pg[:], AF.Identity,
                                             bias=_fp(S["bgam1"])[:,
                                                                  tt:tt + 1])
                        G = SBG.tile([128, CZ], BF, tag="G")
                        nc.vector.tensor_mul(G[:], _bc(va_s[:, tt, :], QC),
                                             pgb[:])
                        Gs[(tt, ii)] = G
                # v1 accumulation dst-major with ii inner: every mW1/WbmF
                # stationary is loaded once per quarter instead of per chunk
                for dst in range(4):
                    v1ps = [PPV.tile([128, CZ], FP, tag="v1p",
                                     name=f"v1p_{qq}_{dst}_{k}")
                            for k in range(CPQ)]
                    for tt in range(4):
                        for ii in range(CPQ):
                            nc.tensor.matmul(v1ps[ii][:],
                                             mW1_s[:, tt, ts(dst, 128)],
                                             Gs[(tt, ii)][:],
                                             start=(tt == 0), stop=False)
                    for ii in range(CPQ):
                        nc.tensor.matmul(v1ps[ii][:],
                                         S["WbmF"][:, ts(dst, 128)],
                                         h1rs[ii][:], start=False, stop=True)
                    for ii in range(CPQ):
                        qcols = ts(ii, CZ)
                        nc.scalar.activation(h2_q[:, dst, qcols], v1ps[ii][:],
                                             GELU_AF,
                                             bias=_fp(S["mb1pp"])[:,
                                                                  dst:dst + 1])
                        sel = S["selF"][:, ts(ii, NCHUNK)][:, 0:CPQ]
                        nc.tensor.matmul(SmP[:], sel, h2_q[:, dst, qcols],
                                         start=(ii == 0 and dst == 0),
                                         stop=(ii == CPQ - 1 and dst == 3))
                        sq2 = SB.tile([128, CZ], FR, tag="sq2")
                        nc.scalar.square(sq2[:], _fp(h2_q[:, dst, qcols]))
                        nc.tensor.matmul(QmP[:], sel, sq2[:],
                                         start=(ii == 0 and dst == 0),
                                         stop=(ii == CPQ - 1 and dst == 3))

            # ---- mixer LN stats (ln/exp table) ----
            # prefetch next quarter's mean/rstd rows ahead of the ln chain
            # so they don't queue behind it on the sync engine
            pf_next = prefetch_mr(qq + 1) if qq + 1 < QSPLIT else None
            qall = ts(qq, CPQ * CZ)
            ln_math(nc, slice(0, CPQ), SmP, QmP, nMq, float(HH), True, RmQ,
                    eps_c)
            nc.sync.dma_start(nMm1[:, :], nMq[0:CPQ, :])
            nc.sync.dma_start(t["scr_rm"][qq * CPQ:(qq + 1) * CPQ, :],
                              RmQ[0:CPQ, :])
            with tc.tile_pool(name="ln_jk", bufs=2, space="PSUM") as JPL:
                _warm_f(JPL, 4)

        # ---- per-quarter softmax ----
        # unnormalized softmax: attention stays exp(logits); the 1/sum is
        # applied once to y_all right before the output projection.  zq is a
        # zero bias column DERIVED FROM RmQ via DVE+DMA: it chains the
        # softmax exp after the mixer-rstd exp so (a) it is not hoisted into
        # B1 and (b) it reuses the exp table set that walrus just loaded for
        # the rstd exp (Ln and Exp live in different sets; unordered they
        # cost two extra ~1.3 us ACT_TABLE_LOADs per quarter).
        zrow = P_bc.tile([1, 1], FP, tag="zrow")
        nc.vector.tensor_scalar_mul(zrow[:], RmQ[0:1, 0:1], 0.0)
        zq = P_bc.tile([128, 1], FP, tag="zq")
        nc.gpsimd.partition_broadcast(zq[64:64 + NH, :], zrow[:])
        attq = log_all[64:64 + NH, qall]
        nc.scalar.activation(attq, attq, AF.Exp, bias=zq[64:64 + NH, :])
        esq = esum_all[64:64 + NH, ts(qq, CPQ * QC)]
        with nc.allow_low_precision(reason="fp32r softmax 1/sum"):
            nc.vector.reduce_sum(
                esq, attq.rearrange("p (c z) -> p c z", z=Z), axis=AX.X)
            nc.vector.reciprocal(esq, _fp(esq))

        # ---- D: v2, rank-1 mean fix, attention apply ----
        # dst-major with ii inner: every mW2F/csmW2b/maskB stationary loads
        # once per quarter instead of per chunk (the fp32r mW2F LDWEIGHTS is
        # the expensive one).  ab and v2p evacuate to bf16 SBUF so the yp
        # multiply runs at DVE 2x instead of the 1x PSUM-operand mode - the
        # old all-PSUM form left the DVE ~2.7 us/chunk behind the PE, and
        # the resulting stalls dropped the PE clock gate mid-phase.
        with tc.tile_pool(name="d_v2", bufs=4, space="PSUM") as PPV2, \
             tc.tile_pool(name="d_ab", bufs=3, space="PSUM") as PPA, \
             tc.tile_pool(name="d_jk", bufs=1, space="PSUM") as JPD, \
             tc.tile_pool(name="d_s4", bufs=4) as SD4, \
             tc.tile_pool(name="d_r2", bufs=4) as SDR, \
             tc.tile_pool(name="d_sb", bufs=4) as SB:
            rmss = []
            for ii in range(CPQ):
                i = qq * CPQ + ii
                rms8t = SDR.tile([64 + NH, CZ], BF, tag="rms8")
                nc.sync.dma_start(rms8t[64:64 + NH, :],
                                  _pbc(t["scr_rm"][i:i + 1, :], NH))
                rmss.append(rms8t)
            attRs = []
            for ii in range(CPQ):
                i = qq * CPQ + ii
                # mixer-LN rstd folded into the 8-row attention tile
                attRt = SD4.tile([64 + NH, CZ], BF, tag="attR")
                attR = attRt[64:64 + NH, :]
                nc.vector.tensor_mul(attR, log_all[64:64 + NH, ts(i, CZ)],
                                     rmss[ii][64:64 + NH, :])
                attRs.append(attR)
            for dst in range(4):
                _warm(JPD, 2)
                v2ps = [PPV2.tile([128, CZ], FP, tag="v2p",
                                  name=f"v2p_{qq}_{dst}_{k}")
                        for k in range(CPQ)]
                for j in range(4):
                    for ii in range(CPQ):
                        nc.tensor.matmul(v2ps[ii][:],
                                         mW2_s[:, j, ts(dst, 128)],
                                         h2_q[:, j, ts(ii, CZ)],
                                         start=(j == 0), stop=False)
                for ii in range(CPQ):
                    nc.tensor.matmul(v2ps[ii][:], S["csmW2b"][:, ts(dst, 128)],
                                     nMm1[:, ts(ii, CZ)],
                                     start=False, stop=True)
                for ii in range(CPQ):
                    i = qq * CPQ + ii
                    ab = PPA.tile([128, CZ], FP, tag="ab")
                    nc.tensor.matmul(ab[:], maskB_hi[:, ts(dst, 128)],
                                     attRs[ii], start=True, stop=True)
                    abs_ = SB.tile([128, CZ], BF, tag="abs")
                    nc.scalar.copy(abs_[:], ab[:])
                    v2s = SB.tile([128, CZ], BF, tag="v2s")
                    nc.scalar.copy(v2s[:], v2ps[ii][:])
                    yp = SB.tile([128, QC, Z], BF, tag="yp")
                    nc.vector.tensor_mul(
                        yp[:], abs_[:].rearrange("p (c z) -> p c z", z=Z),
                        v2s[:].rearrange("p (c z) -> p c z", z=Z))
                    with nc.allow_low_precision(reason="fp32r y"):
                        nc.vector.reduce_sum(
                            y_all[:, dst, i * QC:(i + 1) * QC],
                            yp[:], axis=AX.X)
        pf_cur = pf_next

    # ---------------- OUT (all 32-bit: scales y directly) ----------------
    with tc.tile_pool(name="o_ps", bufs=1, space="PSUM") as PP, \
         tc.tile_pool(name="o_sb", bufs=1) as SB:
        # per-feature gather of the deferred softmax 1/sum, then normalize
        esY = PP.tile([128, 4 * CPC], FP)
        for tt in range(4):
            nc.tensor.matmul(esY[:, ts(tt, CPC)],
                             S["maskT"][:, ts(tt, 128)],
                             esum_all[64:64 + NH, :], start=True, stop=True)
        y_n = SB.tile([128, 4, CPC], FR)
        with nc.allow_low_precision(reason="fp32r y_n"):
            nc.vector.tensor_mul(y_n[:], _fp(y_all[:]),
                                 esY[:].rearrange("p (t c) -> p t c", t=4))
        ops = PP.tile([CPC, HH], FP)
        for j in range(4):
            nc.tensor.matmul(ops[:], y_n[:, j, :], Wo_s[:, j, :],
                             start=(j == 0), stop=False)
        nc.tensor.matmul(ops[:], ones_r[:, 0:CPC], S["boppF"][:],
                         start=False, stop=True)
        osb = SB.tile([CPC, HH], FP)
        nc.scalar.copy(osb[:], ops[:])
        nc.sync.dma_start(t["out"], osb[:])
    stack.close()


def ln_math(nc, rows, St, Qt, Mt, n, negate_mean, Rt, eps_c):
    # St/Qt may live in PSUM (DVE reads at most one PSUM input per op).
    # Mt = (+-)mean; Rt staged as scratch for S^2/n; Qt consumed in place.
    sgn = -1.0 if negate_mean else 1.0
    nc.vector.tensor_scalar_mul(Mt[rows, :], St[rows, :], sgn / n)
    nc.vector.tensor_mul(Rt[rows, :], St[rows, :], Mt[rows, :])
    if negate_mean:
        nc.vector.tensor_add(Qt[rows, :], Qt[rows, :], Rt[rows, :])
    else:
        nc.vector.tensor_sub(Qt[rows, :], Qt[rows, :], Rt[rows, :])
    nc.scalar.activation(Qt[rows, :], Qt[rows, :], AF.Ln,
                         scale=1.0 / n, bias=eps_c[rows, :])
    nc.scalar.activation(Rt[rows, :], Qt[rows, :], AF.Exp, scale=-0.5)


# ======================= host side =======================
_CACHE = {}


def _to_bf16(a):
    return np.ascontiguousarray(a.astype(ml_dtypes.bfloat16))


def _pack(layout_cols, ncol, P, dtype, skip=()):
    A = np.zeros((128, ncol), np.float32)
    for n, (r0, nr, c0, nc_) in layout_cols.items():
        if n in skip:
            continue
        v = P[n]
        assert v.shape == (nr, nc_), (n, v.shape, nr, nc_)
        A[r0:r0 + nr, c0:c0 + nc_] = v
    return _to_bf16(A) if dtype == "b" else A


def _host_prep(inp):
    g = {k: np.ascontiguousarray(np.asarray(v, np.float32)) for k, v in inp.items()}
    P = {}
    P["Bcat"] = np.concatenate([g["B_q"], g["B_q"], g["B_v"], g["B_v"]], 1)
    qb = np.zeros((1, 128), np.float32)
    qb[0, 32:64] = 0.25
    qb[0, 96:128] = 0.25
    P["qb"] = qb
    Wqv = np.zeros((128, 128), np.float32)
    Wqv[0:64, 0:64] = -np.concatenate([g["Wqe"][:32], g["Wqe"][32:]], 0)
    Wqv[64:128, 64:128] = -np.concatenate([g["Wve"][:32], g["Wve"][32:]], 0)
    P["Wqv"] = Wqv
    bqp = (g["bqe"] @ g["Wq"] + g["bq"])[None, :]
    P["bqc"] = np.ascontiguousarray(bqp.reshape(4, 128).T)  # 0.125 is in kv_s
    P["vW1b"] = g["vW1"]
    P["vb1p"] = (g["bve"] @ g["vW1"] + g["vb1"])[:, None]
    vW2p = g["vg"][:, None] * g["vW2"]
    vb2p = g["vbn"] @ g["vW2"] + g["vb2"]
    P["WgamF"] = vW2p[:, :HH]
    Wbeta, bbeta = vW2p[:, HH:], vb2p[HH:]
    P["bcol"] = None  # filled below
    bgam1 = np.ascontiguousarray((1.0 + vb2p[:HH]).reshape(4, 128).T)
    P["mW1"] = g["mW1"]
    P["WbmF"] = Wbeta @ g["mW1"]
    mb1pp = np.ascontiguousarray(
        (bbeta @ g["mW1"] + g["mb1"]).reshape(4, 128).T)
    mW2p = g["mg"][:, None] * g["mW2"]
    mb2p = g["mbn"] @ g["mW2"] + g["mb2"]
    P["mW2F"] = mW2p
    P["csmW2b"] = mW2p.sum(0)[None, :]
    P["WoF"] = g["Wo"]
    P["boppF"] = (mb2p @ g["Wo"] + g["bo"])[None, :]
    P["Wq"] = g["Wq"]
    P["WkF"], P["WvF"] = g["Wk"], g["Wv"]
    P["bkv"] = np.concatenate([g["bk"][None, :], g["bv"][None, :]], 1)
    P["bcol"] = np.concatenate([bgam1, mb1pp], 1)
    for wn in ("mW1", "mW2F", "WoF"):
        P[wn] = np.ascontiguousarray(
            P[wn].reshape(4, 128, HH).transpose(1, 0, 2).reshape(128, 4 * HH))
    P["onec"] = np.ones((128, 1), np.float32)
    P["oner"] = np.ones((1, CZ), np.float32)
    selS = np.zeros((128, NCHUNK, NCHUNK), np.float32)
    for i in range(NCHUNK):
        selS[:, i, i] = 1.0
    P["selS"] = np.ascontiguousarray(selS.reshape(128, NCHUNK * NCHUNK))
    P["selF"] = P["selS"]
    P["eyeZ"] = np.ascontiguousarray(np.tile(np.eye(Z, dtype=np.float32),
                                             (1, QC)))
    mS = np.zeros((128, 4, NH), np.float32)
    for tt in range(4):
        for p in range(128):
            mS[p, tt, 2 * tt + p // 64] = 1.0
    P["maskS"] = np.ascontiguousarray(mS.reshape(128, 32))
    P["maskB"] = np.zeros((NH, HH), np.float32)
    for h in range(NH):
        P["maskB"][h, h * H:(h + 1) * H] = 1.0
    mT = np.zeros((NH, 4, 128), np.float32)
    for tt in range(4):
        for p in range(128):
            mT[2 * tt + p // 64, tt, p] = 1.0
    P["maskT"] = np.ascontiguousarray(mT.reshape(NH, 4 * 128))
    return P, g


def make_in_maps(P, g):
    base_a = _pack(CPA_COLS, CPA_NCOL, P, "f", skip=("xp",))
    base_b = _pack(CPB_COLS, CPB_NCOL, P, "f", skip=("aT",))
    ww = _pack(WPW_COLS, WPW_NCOL, P, "b")
    cd = _pack(CPD_COLS, CPD_NCOL, P, "f")
    xT_full = np.ascontiguousarray(g["inputs"].reshape(B * C, D).T)
    in_maps = []
    for core in range(NCORE):
        b = core // (NCORE // B)
        A = base_a.copy()
        r0, nr, c0, ncol = CPA_COLS["xp"]
        A[r0:r0 + nr, c0:c0 + ncol] = np.concatenate(
            [xT_full[:, core * CPC:(core + 1) * CPC], g["p"][b].T], 1)
        Bm = base_b.copy()
        r0, nr, c0, ncol = CPB_COLS["aT"]
        Bm[r0:r0 + nr, c0:c0 + ncol] = g["a"][b].T
        in_maps.append({"cpk_a": A, "cpk_b": Bm, "wpk_w": ww, "cpk_d": cd})
    return in_maps


def kernel(**inputs):
    P, g = _host_prep(inputs)
    if "nc" not in _CACHE:
        _CACHE["nc"] = build_kernel()
    nc = _CACHE["nc"]
    in_maps = make_in_maps(P, g)
    res = run_bass_kernel_spmd(nc, in_maps, core_ids=list(range(NCORE)))
    outs = [res.results[i]["out"] for i in range(NCORE)]
    return np.concatenate(outs, 0).reshape(B, C, HH).astype(np.float32)


if __name__ == "__main__":
    import reference
    inp = {k: np.asarray(v) for k, v in reference.setup_inputs().items()}
    got = kernel(**inp)
    exp = np.asarray(reference.reference(**reference.setup_inputs()))
    err = np.abs(got - exp)
    scale = float(np.sqrt((exp ** 2).mean()))
    print("max abs err:", err.max(), " scaled:", err.max() / scale)


# revision 27
# speedup vs baseline: 1.0257x; 1.0257x over previous
"""Trainium2 Bass kernel for nn_EquivariantCrossAttention.

Sharding: batch*query rows (2*256=512) split across 8 cores (64 queries each,
cores 0-3 -> batch 0, cores 4-7 -> batch 1). k/v/a replicated per batch.

Per-core layout: feature-on-partition, (c,z) flattened on the free dim.
64 queries x 128 latents = 8192 free columns, processed in 16 chunks of 512.

Key implementation notes (validated vs reference in a numpy quantization
model and on hardware):
  - The PE's float32r mode is ~10-bit-mantissa round-to-nearest (tf32-like).
    A 4-byte LDWEIGHTS costs ~185 ns and cannot hide behind the previous
    matmul, so weight-switching fp32r matmuls run at 327-427 ns vs 234 ns
    for weight-reusing ones.  bf16 weights+operands stream at ~240-265 ns.
    Walrus forbids mixing 32-bit and 16-bit matmul operands, so chains flip
    to bf16 wholesale.  Per the numpy error model, bf16 is near-free for
    the B1 q/k/logits chain, the mW1/G chain, and masks/selectors, but the
    mW2 output chain (no LN downstream) and the pg/Wbm/h1r pair cost real
    accuracy - those stay fp32r, as do the RFF phase path, logits + softmax
    (fp32), LN statistics, and the output projection.
  - Phase order is B0 -> PRE -> B1: engine queues are strict FIFO, so PRE's
    ACT ops (waiting on the late weight DMA) must not queue ahead of B0's
    sin chain.  Constants arrive in four DMAs ordered by first use.
  - HAM warm filler (junk matmuls) is load-bearing: without it the PE clock
    gate drops to 4/8 in ACT/DVE-bound stretches and can stay cold for tens
    of microseconds even under dense load afterwards.  Junk pools use
    bufs=2 so filler pipelines at ~235 ns instead of serializing at ~770.
  - RFF dense biases folded into downstream weights (bqe->bq', bve->vb1').
  - LayerNorm gain/bias folded into the following matmul (W'=g*W, b'=bn@W+b).
  - vLN mean removed with one subtract; vLN rstd multiplied into h1 once
    (h1r) and commuted through the mW1/Wbm matmuls.
  - mixer-LN mean via rank-1 matmul fold (csmW2 x -mean); mixer rstd folded
    into the 8-row attention tile (attR) instead of the 128-row v2.
  - rstd = exp(-0.5*ln(var+eps)) so LN and softmax share the natural_log_exp
    activation-table set (hardware Rsqrt is forbidden; table swaps cost 2.7us).
  - FiLM: G = va*(pg + bgam1), the bias applied as a per-partition ACT bias
    while evacuating pg from PSUM (cheaper than the old amwT/eyeZ matmul).
  - LN statistics accumulated straight into multi-partition PSUM rows via
    one-hot selector matmuls (selS bf16 / selF fp32), read by ln_math.
  - v3 bias folded into the output projection bias via softmax-sum=1.
  - Softmax without max subtraction; exp+normalize per quarter, in place on
    log_all, inside the Ln/Exp window.  The exp is pinned into its quarter
    via a zero bias column derived from h2_q; otherwise the scheduler hoists
    it into B1, paying two gelu<->ln_exp ACT-table reloads there per quarter.
"""
import sys
import numpy as np

for _p in ("/opt/trn_rl_repo",):
    if _p not in sys.path:
        sys.path.insert(0, _p)

import ml_dtypes
import concourse.bass as bass
import concourse.tile as tile
from concourse import bacc, mybir
from concourse.bass_utils import run_bass_kernel_spmd

FP = mybir.dt.float32
FR = mybir.dt.float32r
BF = mybir.dt.bfloat16
AF = mybir.ActivationFunctionType
OP = mybir.AluOpType
AX = mybir.AxisListType
ts = bass.ts

GELU_AF = AF.Gelu_apprx_tanh  # sim_test overrides (sim lacks gelu)

B, C, Z, D = 2, 256, 128, 3
H, NH, HH = 64, 8, 512
EPS = 1e-5
NCORE = 8
CPC = (B * C) // NCORE          # 64 queries per core
QC = 4                          # queries per chunk
CZ = QC * Z                     # 512 free columns per chunk
NCHUNK = CPC // QC              # 16
QSPLIT = 4                      # process h2 in quarters (SBUF)
CPQ = NCHUNK // QSPLIT          # 4 chunks per quarter
CZALL = CPC * Z                 # 8192


def _fp(ap):
    """Read a float32r AP as plain fp32 (same bits) for DVE/ACT consumers."""
    return ap.bitcast(FP)


# packed-constant layouts: (name, base_row, nrows, ncols)
# cpk_a (fp32, DMA 1): everything B0 needs + ACT columns + fp32 selectors
CPA_LAYOUT = [
    ("xp", 0, D + 1, CPC + Z), ("Bcat", 0, D + 1, 128),
    ("Wqv", 0, 128, 128), ("bqc", 0, 128, 4),
    ("vb1p", 0, H, 1), ("bcol", 0, 128, 8), ("onec", 0, 128, 1),
    ("oner", 0, 1, CZ), ("selF", 0, 128, NCHUNK * NCHUNK),
]
# cpk_b (fp32, DMA 2): PRE k/v chain + the accuracy-critical B2 weights
CPB_LAYOUT = [
    ("aT", 0, H, Z), ("WkF", 0, H, HH), ("WvF", 0, H, HH),
    ("bkv", 0, 1, 2 * HH), ("WgamF", 0, H, HH), ("WbmF", 0, H, HH),
]
# wpk_w (bf16, DMA 3): B1 q chain + masks + mW1 chain
WPW_LAYOUT = [
    ("Wq", 0, H, HH), ("vW1b", 64, H, H), ("maskS", 0, 128, 32),
    ("selS", 0, 128, NCHUNK * NCHUNK),
    ("maskB", 64, NH, HH), ("mW1", 0, 128, 4 * HH), ("csmW2b", 0, 1, HH),
]
# cpk_d (fp32, DMA 4): mW2 chain + output projection (quarters onward)
CPD_LAYOUT = [
    ("mW2F", 0, 128, 4 * HH), ("maskT", 64, NH, 4 * 128),
    ("WoF", 0, 128, 4 * HH), ("boppF", 0, 1, HH),
]


def _cols(layout):
    d = {}
    c = 0
    for n, r, nr, ncol in layout:
        d[n] = (r, nr, c, ncol)
        c += ncol
    return d, c


CPA_COLS, CPA_NCOL = _cols(CPA_LAYOUT)
CPB_COLS, CPB_NCOL = _cols(CPB_LAYOUT)
WPW_COLS, WPW_NCOL = _cols(WPW_LAYOUT)
CPD_COLS, CPD_NCOL = _cols(CPD_LAYOUT)


def _bc(ap, outer):
    """[P,n] -> [P,outer,n] with stride-0 outer dim (broadcast over queries)."""
    return bass.AP(tensor=ap.tensor, offset=ap.offset,
                   ap=[ap.ap[0], [0, outer]] + list(ap.ap[1:]))


def _pbc(ap, nparts):
    """[1,n] -> [nparts,n] partition-broadcast AP (stride-0 partitions; DMA only)."""
    return bass.AP(tensor=ap.tensor, offset=ap.offset,
                   ap=[[0, nparts]] + list(ap.ap[1:]))


def _bc_inner(ap, inner):
    """[P,n] -> [P,n,inner] with stride-0 inner dim."""
    return bass.AP(tensor=ap.tensor, offset=ap.offset,
                   ap=list(ap.ap) + [[0, inner]])


def build_kernel():
    nc = bacc.Bacc("TRN2", target_bir_lowering=False, debug=False,
                   num_devices=NCORE)

    t = {}
    t["cpk_a"] = nc.dram_tensor("cpk_a", [128, CPA_NCOL], FR,
                                kind="ExternalInput").ap()
    t["cpk_b"] = nc.dram_tensor("cpk_b", [128, CPB_NCOL], FR,
                                kind="ExternalInput").ap()
    t["wpk_w"] = nc.dram_tensor("wpk_w", [128, WPW_NCOL], BF,
                                kind="ExternalInput").ap()
    t["cpk_d"] = nc.dram_tensor("cpk_d", [128, CPD_NCOL], FR,
                                kind="ExternalInput").ap()
    t["out"] = nc.dram_tensor("out", [CPC, HH], FP, kind="ExternalOutput").ap()

    with tile.TileContext(nc) as tc:
        body(tc, t)
    nc.finalize()
    return nc


def body(tc, t):
    nc = tc.nc
    t = dict(t)
    t["scr_mr"] = nc.dram_tensor("scr_mr", [NCHUNK, 2, CZ], BF,
                                 kind="Internal").ap()
    t["scr_rm"] = nc.dram_tensor("scr_rm", [NCHUNK, CZ], BF, kind="Internal").ap()
    import contextlib
    stack = contextlib.ExitStack()
    P_const = stack.enter_context(tc.tile_pool(name="const", bufs=1))
    P_big = stack.enter_context(tc.tile_pool(name="big", bufs=1))

    cpa = P_const.tile([128, CPA_NCOL], FR, tag="cpa")
    cpb = P_const.tile([128, CPB_NCOL], FR, tag="cpb")
    wpw = P_const.tile([128, WPW_NCOL], BF, tag="wpw")
    cpd = P_const.tile([128, CPD_NCOL], FR, tag="cpd")
    # ordered by first use: B0 needs only cpa; PRE cpb; B1 wpw; quarters cpd
    nc.sync.dma_start(cpa[:], t["cpk_a"])
    nc.sync.dma_start(cpb[:], t["cpk_b"])
    nc.sync.dma_start(wpw[:], t["wpk_w"])
    nc.sync.dma_start(cpd[:], t["cpk_d"])

    S = {}
    for tl, cols in ((cpa, CPA_COLS), (cpb, CPB_COLS), (wpw, WPW_COLS),
                     (cpd, CPD_COLS)):
        for n, (r0, nr, c0, ncol) in cols.items():
            S[n] = tl[r0:r0 + nr, c0:c0 + ncol]
    S["xT"] = S["xp"][:, 0:CPC]
    S["pT"] = S["xp"][:, CPC:CPC + Z]
    S["bgam1"], S["mb1pp"] = S["bcol"][:, 0:4], S["bcol"][:, 4:8]
    maskB_hi = S["maskB"]
    mW1_s = S["mW1"].rearrange("p (j n) -> p j n", j=4)
    mW2_s = S["mW2F"].rearrange("p (j n) -> p j n", j=4)
    Wo_s = S["WoF"].rearrange("p (j n) -> p j n", j=4)
    ones_r = S["oner"]          # [1,CZ]  float32r ones

    eps_c = P_const.tile([128, 1], FP)
    nc.vector.memset(eps_c[:], EPS)
    wj = P_const.tile([128, CZ], BF)
    nc.vector.memset(wj[:], 0.0)
    wjf = P_const.tile([128, CZ], FP)
    nc.vector.memset(wjf[:], 0.0)

    def _warm(pool, n, ncols=CZ):
        """Throwaway bf16 matmuls (~235 ns at N=512 warm): keep the PE HAM
        clock-gate at 8/8 wherever real matmul density dips.  Load-bearing:
        once the gate drops to 4/8 it can stay cold for tens of us even
        under a saturated half-clocked matmul stream."""
        for _r in range(n):
            jp = pool.tile([128, CZ], FP, tag="junk")
            nc.tensor.matmul(jp[:, 0:ncols], wj[:, 0:128], wj[:, 0:ncols],
                             start=True, stop=True)

    def _warm_f(pool, n):
        """fp32 filler (~1.2 us each): for multi-us PE-idle windows."""
        for _r in range(n):
            jp = pool.tile([128, CZ], FP, tag="junk")
            nc.tensor.matmul(jp[:], wjf[:, 0:128], wjf[:],
                             start=True, stop=True)

    # dense burst while the cpk_a DMA streams in
    with tc.tile_pool(name="warm_ps", bufs=2, space="PSUM") as WP:
        _warm(WP, 16)

    # persistent buffers
    h1_all = P_big.tile([H, CZALL], BF)         # B1 gelu output
    log_all = P_big.tile([64 + NH, CZALL], FP)  # rows 64-71: logits -> att
    y_all = P_big.tile([128, 4, CPC], FR)
    MvRv = P_big.tile([NCHUNK, 2, CZ], BF)  # vLN mean | rstd, interleaved
    nMq = P_big.tile([CPQ, CZ], BF)
    RmQ = P_big.tile([CPQ, CZ], BF)
    nMm1 = P_big.tile([1, CPQ * CZ], BF)   # one quarter's negated means, row form
    esum_all = P_big.tile([64 + NH, CPC], FR)  # softmax 1/sum, rows 64-71
    kv_s = P_big.tile([128, 4, Z], BF)
    va_s = P_big.tile([128, 4, Z], FP)

    # ---------------- B0: inv -> RFF -> ie (sin); needs only cpk_a --------
    with tc.tile_pool(name="iep", bufs=1) as ie_pool:
        ie_all = ie_pool.tile([128, CZALL], BF)
        with tc.tile_pool(name="b0_ps", bufs=2, space="PSUM") as PP, \
             tc.tile_pool(name="b0_jk", bufs=2, space="PSUM") as JP0, \
             tc.tile_pool(name="pre_ps", bufs=1, space="PSUM") as PPP, \
             tc.tile_pool(name="pre_sb", bufs=1) as PSB, \
             tc.tile_pool(name="b0_sb", bufs=3) as SB:
            RC = 12582912.0  # 1.5 * 2^23: fp32 add rounds to nearest integer
            for i in range(NCHUNK):
                _warm(JP0, 4)
                cols = ts(i, CZ)
                inv = SB.tile([D + 1, QC, Z], FR, tag="inv")
                nc.vector.tensor_sub(
                    inv[:], _bc_inner(_fp(S["xT"])[:, ts(i, QC)], Z),
                    _bc(_fp(S["pT"])[:, :], QC))
                # rows: [m_q, m_q+0.25, m_v, m_v+0.25]; the 0.25-turn
                # cos shift rides Bcat's 4th row against inv's constant-1
                # 4th row (xT row 3 = 1, pT row 3 = 0), so one K=4 matmul
                # yields phase+shift and the sin argument stays in +-pi
                mm = PP.tile([128, CZ], FP, tag="mm")
                nc.tensor.matmul(mm[:], S["Bcat"][:], inv[:], start=True,
                                 stop=True)
                r1 = SB.tile([128, CZ], FP, tag="r1")
                nc.scalar.activation(r1[:], mm[:], AF.Copy, bias=RC)
                fr = SB.tile([128, CZ], FP, tag="fr")
                nc.vector.scalar_tensor_tensor(fr[:], r1[:], RC, mm[:],
                                               op0=OP.subtract,
                                               op1=OP.subtract)
                F = SB.tile([128, CZ], FR, tag="F")
                nc.scalar.activation(F[:], fr[:], AF.Sin,
                                     scale=float(2 * np.pi))
                ieps = PP.tile([128, CZ], FP, tag="ieps")
                nc.tensor.matmul(ieps[:], S["Wqv"][:], F[:],
                                 start=True, stop=True)
                nc.scalar.copy(ie_all[:, cols], ieps[:])

            # ---- PRE: k, va, bqkT (needs cpk_b; overlaps B0 on the PE) ----
            bkv = S["bkv"].rearrange("p (k n) -> p k n", k=2)
            # kv_s carries the 1/sqrt(H)=0.125 attention scale
            for dst_s, W_n, bi, scl in [(kv_s, "WkF", 0, 0.125),
                                        (va_s, "WvF", 1, 1.0)]:
                for tt in range(4):
                    ps = PPP.tile([128, Z], FP, tag="kv")
                    nc.tensor.matmul(ps[:], S[W_n][:, ts(tt, 128)],
                                     S["aT"][:], start=True, stop=False)
                    nc.tensor.matmul(ps[:], bkv[:, bi, ts(tt, 128)],
                                     ones_r[:, 0:Z], start=False, stop=True)
                    nc.scalar.activation(dst_s[:, tt, :], ps[:], AF.Copy,
                                         scale=scl)

        # ---- B1: q/logits, h1, vLN stats into PSUM (gelu) ----
        with tc.tile_pool(name="b1_st", bufs=1, space="PSUM") as PPS1:
            SvP = PPS1.tile([NCHUNK, CZ], FP)
            QvP = PPS1.tile([NCHUNK, CZ], FP)
            with tc.tile_pool(name="b1_ps", bufs=1, space="PSUM") as PP, \
                 tc.tile_pool(name="b1_qps", bufs=3, space="PSUM") as PPQ, \
                 tc.tile_pool(name="b1_jk", bufs=1, space="PSUM") as JP1, \
                 tc.tile_pool(name="b1_ek", bufs=8) as SBE, \
                 tc.tile_pool(name="b1_sb", bufs=2) as SB:
                for i in range(NCHUNK):
                    _warm(JP1, 1)
                    cols = ts(i, CZ)
                    # h1 path first: its gelu/square run on ACT while the PE
                    # works through the q-path matmuls below
                    h1ps = PP.tile([H, CZ], FP, tag="h1ps")
                    nc.tensor.matmul(h1ps[:], S["vW1b"][:],
                                     ie_all[64:128, cols],
                                     start=True, stop=True)
                    qpss = []
                    for tt in range(4):
                        qps = PPQ.tile([128, CZ], FP, tag="qps")
                        nc.tensor.matmul(qps[:], S["Wq"][:, ts(tt, 128)],
                                         ie_all[0:64, cols],
                                         start=True, stop=True)
                        qpss.append(qps)
                    nc.scalar.activation(h1_all[:, cols], h1ps[:], GELU_AF,
                                         bias=_fp(S["vb1p"])[:])
                    sq = SB.tile([H, CZ], BF, tag="sq")
                    nc.scalar.square(sq[:], h1_all[:, cols])
                    eks = []
                    for tt in range(4):
                        # q bias folds into the PSUM evacuation; bf16 SBUF
                        # operands put the k-multiply in the DVE 2x mode
                        qsb = SBE.tile([128, CZ], BF, tag="qsb")
                        nc.scalar.activation(qsb[:], qpss[tt][:], AF.Identity,
                                             bias=_fp(S["bqc"][:,
                                                              tt:tt + 1]))
                        ek = SBE.tile([128, CZ], BF, tag="ek")
                        nc.vector.tensor_mul(ek[:], qsb[:],
                                             _bc(kv_s[:, tt, :], QC))
                        eks.append(ek)
                    lps = PP.tile([NH, CZ], FP, tag="lps")
                    for tt in range(4):
                        nc.tensor.matmul(lps[:], S["maskS"][:, ts(tt, NH)],
                                         eks[tt][:], start=(tt == 0),
                                         stop=(tt == 3))
                    nc.vector.tensor_copy(log_all[64:64 + NH, cols], lps[:])
                    sel = S["selS"][0:64, ts(i, NCHUNK)]
                    nc.tensor.matmul(SvP[:], sel, h1_all[:, cols],
                                     start=(i == 0), stop=(i == NCHUNK - 1))
                    nc.tensor.matmul(QvP[:], sel, sq[:],
                                     start=(i == 0), stop=(i == NCHUNK - 1))

            # ---- C1: vLN rstd (ln/exp); stats read from PSUM in place ----
            ln_math(nc, slice(0, NCHUNK), SvP, QvP, MvRv[:, 0, :], float(H),
                    False, MvRv[:, 1, :], eps_c)
            nc.sync.dma_start(t["scr_mr"], MvRv[:])
            with tc.tile_pool(name="c1_jk", bufs=2, space="PSUM") as JPC:
                _warm_f(JPC, 4)

    # ---------------- quarters: B2 (gelu) -> ln+softmax -> D -------------
    h2_pool = stack.enter_context(tc.tile_pool(name="h2p", bufs=1))
    h2_q = h2_pool.tile([128, 4, CPQ * CZ], FR)
    P_bc = stack.enter_context(tc.tile_pool(name="bcast", bufs=8))

    def prefetch_mr(qq):
        """Broadcast each chunk's vLN (mean|rstd) row pair to H rows."""
        pf = []
        for ii in range(CPQ):
            i = qq * CPQ + ii
            mr = P_bc.tile([H, 2, CZ], BF, tag="mr")
            src = t["scr_mr"][i, :, :]
            nc.sync.dma_start(mr[:], bass.AP(tensor=src.tensor,
                                             offset=src.offset,
                                             ap=[[0, H]] + list(src.ap)))
            pf.append(mr)
        return pf

    pf_cur = prefetch_mr(0)
    for qq in range(QSPLIT):
        with tc.tile_pool(name="b2_st", bufs=1, space="PSUM") as PPS:
            SmP = PPS.tile([CPQ, CZ], FP, tag="SmP")
            QmP = PPS.tile([CPQ, CZ], FP, tag="QmP")
            with tc.tile_pool(name="qb_jk", bufs=2, space="PSUM") as JPQ:
                _warm(JPQ, 10)
            with tc.tile_pool(name="b2_pg", bufs=2, space="PSUM") as PPG, \
                 tc.tile_pool(name="b2_v1", bufs=4, space="PSUM") as PPV, \
                 tc.tile_pool(name="b2_s4", bufs=4) as SB4, \
                 tc.tile_pool(name="b2_sb", bufs=3) as SB, \
                 tc.tile_pool(name="b2_g4", bufs=16) as SBG:
                h1rs = []
                for ii in range(CPQ):
                    cols = ts(qq * CPQ + ii, CZ)
                    h1c = SB.tile([H, CZ], FP, tag="h1c")
                    nc.vector.tensor_sub(h1c[:], h1_all[:, cols],
                                         pf_cur[ii][:, 0, :])
                    h1r = SB4.tile([H, CZ], FR, tag="h1r")
                    nc.vector.tensor_mul(h1r[:], h1c[:], pf_cur[ii][:, 1, :])
                    h1rs.append(h1r)
                # pg/G for the whole quarter, tt-major: each WgamF slice's
                # 4-byte LDWEIGHTS is amortized over the 4 chunks
                Gs = {}
                for tt in range(4):
                    for ii in range(CPQ):
                        pg = PPG.tile([128, CZ], FP, tag="pg")
                        nc.tensor.matmul(pg[:], S["WgamF"][:, ts(tt, 128)],
                                         h1rs[ii][:], start=True, stop=True)
                        # FiLM bias folded in while evacuating PSUM
                        pgb = SB.tile([128, CZ], FP, tag="pgb")
                        nc.scalar.activation(pgb[:], pg[:], AF.Identity,
                                             bias=_fp(S["bgam1"])[:,
                                                                  tt:tt + 1])
                        G = SBG.tile([128, CZ], BF, tag="G")
                        nc.vector.tensor_mul(G[:], _bc(va_s[:, tt, :], QC),
                                             pgb[:])
                        Gs[(tt, ii)] = G
                # v1 accumulation dst-major with ii inner: every mW1/WbmF
                # stationary is loaded once per quarter instead of per chunk
                for dst in range(4):
                    v1ps = [PPV.tile([128, CZ], FP, tag="v1p",
                                     name=f"v1p_{qq}_{dst}_{k}")
                            for k in range(CPQ)]
                    for tt in range(4):
                        for ii in range(CPQ):
                            nc.tensor.matmul(v1ps[ii][:],
                                             mW1_s[:, tt, ts(dst, 128)],
                                             Gs[(tt, ii)][:],
                                             start=(tt == 0), stop=False)
                    for ii in range(CPQ):
                        nc.tensor.matmul(v1ps[ii][:],
                                         S["WbmF"][:, ts(dst, 128)],
                                         h1rs[ii][:], start=False, stop=True)
                    for ii in range(CPQ):
                        qcols = ts(ii, CZ)
                        nc.scalar.activation(h2_q[:, dst, qcols], v1ps[ii][:],
                                             GELU_AF,
                                             bias=_fp(S["mb1pp"])[:,
                                                                  dst:dst + 1])
                        sel = S["selF"][:, ts(ii, NCHUNK)][:, 0:CPQ]
                        nc.tensor.matmul(SmP[:], sel, h2_q[:, dst, qcols],
                                         start=(ii == 0 and dst == 0),
                                         stop=(ii == CPQ - 1 and dst == 3))
                        sq2 = SB.tile([128, CZ], FR, tag="sq2")
                        nc.scalar.square(sq2[:], _fp(h2_q[:, dst, qcols]))
                        nc.tensor.matmul(QmP[:], sel, sq2[:],
                                         start=(ii == 0 and dst == 0),
                                         stop=(ii == CPQ - 1 and dst == 3))

            # ---- mixer LN stats (ln/exp table) ----
            # prefetch next quarter's mean/rstd rows ahead of the ln chain
            # so they don't queue behind it on the sync engine
            pf_next = prefetch_mr(qq + 1) if qq + 1 < QSPLIT else None
            qall = ts(qq, CPQ * CZ)
            ln_math(nc, slice(0, CPQ), SmP, QmP, nMq, float(HH), True, RmQ,
                    eps_c)
            nc.sync.dma_start(nMm1[:, :], nMq[0:CPQ, :])
            nc.sync.dma_start(t["scr_rm"][qq * CPQ:(qq + 1) * CPQ, :],
                              RmQ[0:CPQ, :])
            with tc.tile_pool(name="ln_jk", bufs=2, space="PSUM") as JPL:
                _warm_f(JPL, 4)

        # ---- per-quarter softmax ----
        # unnormalized softmax: attention stays exp(logits); the 1/sum is
        # applied once to y_all right before the output projection.  zq is a
        # zero bias column DERIVED FROM RmQ via DVE+DMA: it chains the
        # softmax exp after the mixer-rstd exp so (a) it is not hoisted into
        # B1 and (b) it reuses the exp table set that walrus just loaded for
        # the rstd exp (Ln and Exp live in different sets; unordered they
        # cost two extra ~1.3 us ACT_TABLE_LOADs per quarter).
        zrow = P_bc.tile([1, 1], FP, tag="zrow")
        nc.vector.tensor_scalar_mul(zrow[:], RmQ[0:1, 0:1], 0.0)
        zq = P_bc.tile([128, 1], FP, tag="zq")
        nc.gpsimd.partition_broadcast(zq[64:64 + NH, :], zrow[:])
        attq = log_all[64:64 + NH, qall]
        nc.scalar.activation(attq, attq, AF.Exp, bias=zq[64:64 + NH, :])
        esq = esum_all[64:64 + NH, ts(qq, CPQ * QC)]
        with nc.allow_low_precision(reason="fp32r softmax 1/sum"):
            nc.vector.reduce_sum(
                esq, attq.rearrange("p (c z) -> p c z", z=Z), axis=AX.X)
            nc.vector.reciprocal(esq, _fp(esq))

        # ---- D: v2, rank-1 mean fix, attention apply ----
        # dst-major with ii inner: every mW2F/csmW2b/maskB stationary loads
        # once per quarter instead of per chunk (the fp32r mW2F LDWEIGHTS is
        # the expensive one).  ab and v2p evacuate to bf16 SBUF so the yp
        # multiply runs at DVE 2x instead of the 1x PSUM-operand mode - the
        # old all-PSUM form left the DVE ~2.7 us/chunk behind the PE, and
        # the resulting stalls dropped the PE clock gate mid-phase.
        with tc.tile_pool(name="d_v2", bufs=4, space="PSUM") as PPV2, \
             tc.tile_pool(name="d_ab", bufs=3, space="PSUM") as PPA, \
             tc.tile_pool(name="d_jk", bufs=1, space="PSUM") as JPD, \
             tc.tile_pool(name="d_s4", bufs=4) as SD4, \
             tc.tile_pool(name="d_r2", bufs=4) as SDR, \
             tc.tile_pool(name="d_sb", bufs=4) as SB:
            rmss = []
            for ii in range(CPQ):
                i = qq * CPQ + ii
                rms8t = SDR.tile([64 + NH, CZ], BF, tag="rms8")
                nc.sync.dma_start(rms8t[64:64 + NH, :],
                                  _pbc(t["scr_rm"][i:i + 1, :], NH))
                rmss.append(rms8t)
            _warm(JPD, 4)
            attRs = []
            for ii in range(CPQ):
                i = qq * CPQ + ii
                # mixer-LN rstd folded into the 8-row attention tile
                attRt = SD4.tile([64 + NH, CZ], BF, tag="attR")
                attR = attRt[64:64 + NH, :]
                nc.vector.tensor_mul(attR, log_all[64:64 + NH, ts(i, CZ)],
                                     rmss[ii][64:64 + NH, :])
                attRs.append(attR)
            for dst in range(4):
                _warm(JPD, 2)
                v2ps = [PPV2.tile([128, CZ], FP, tag="v2p",
                                  name=f"v2p_{qq}_{dst}_{k}")
                        for k in range(CPQ)]
                for j in range(4):
                    for ii in range(CPQ):
                        nc.tensor.matmul(v2ps[ii][:],
                                         mW2_s[:, j, ts(dst, 128)],
                                         h2_q[:, j, ts(ii, CZ)],
                                         start=(j == 0), stop=False)
                for ii in range(CPQ):
                    nc.tensor.matmul(v2ps[ii][:], S["csmW2b"][:, ts(dst, 128)],
                                     nMm1[:, ts(ii, CZ)],
                                     start=False, stop=True)
                for ii in range(CPQ):
                    i = qq * CPQ + ii
                    ab = PPA.tile([128, CZ], FP, tag="ab")
                    nc.tensor.matmul(ab[:], maskB_hi[:, ts(dst, 128)],
                                     attRs[ii], start=True, stop=True)
                    abs_ = SB.tile([128, CZ], BF, tag="abs")
                    nc.scalar.copy(abs_[:], ab[:])
                    v2s = SB.tile([128, CZ], BF, tag="v2s")
                    nc.scalar.copy(v2s[:], v2ps[ii][:])
                    yp = SB.tile([128, QC, Z], BF, tag="yp")
                    nc.vector.tensor_mul(
                        yp[:], abs_[:].rearrange("p (c z) -> p c z", z=Z),
                        v2s[:].rearrange("p (c z) -> p c z", z=Z))
                    with nc.allow_low_precision(reason="fp32r y"):
                        nc.vector.reduce_sum(
                            y_all[:, dst, i * QC:(i + 1) * QC],
                            yp[:], axis=AX.X)
        pf_cur = pf_next

    # ---------------- OUT (all 32-bit: scales y directly) ----------------
    with tc.tile_pool(name="o_ps", bufs=1, space="PSUM") as PP, \
         tc.tile_pool(name="o_sb", bufs=1) as SB:
        # per-feature gather of the deferred softmax 1/sum, then normalize
        esY = PP.tile([128, 4 * CPC], FP)
        for tt in range(4):
            nc.tensor.matmul(esY[:, ts(tt, CPC)],
                             S["maskT"][:, ts(tt, 128)],
                             esum_all[64:64 + NH, :], start=True, stop=True)
        y_n = SB.tile([128, 4, CPC], FR)
        with nc.allow_low_precision(reason="fp32r y_n"):
            nc.vector.tensor_mul(y_n[:], _fp(y_all[:]),
                                 esY[:].rearrange("p (t c) -> p t c", t=4))
        ops = PP.tile([CPC, HH], FP)
        for j in range(4):
            nc.tensor.matmul(ops[:], y_n[:, j, :], Wo_s[:, j, :],
                             start=(j == 0), stop=False)
        nc.tensor.matmul(ops[:], ones_r[:, 0:CPC], S["boppF"][:],
                         start=False, stop=True)
        osb = SB.tile([CPC, HH], FP)
        nc.scalar.copy(osb[:], ops[:])
        nc.sync.dma_start(t["out"], osb[:])
    stack.close()


def ln_math(nc, rows, St, Qt, Mt, n, negate_mean, Rt, eps_c):
    # St/Qt may live in PSUM (DVE reads at most one PSUM input per op).
    # Mt = (+-)mean; Rt staged as scratch for S^2/n; Qt consumed in place.
    sgn = -1.0 if negate_mean else 1.0
    nc.vector.tensor_scalar_mul(Mt[rows, :], St[rows, :], sgn / n)
    nc.vector.tensor_mul(Rt[rows, :], St[rows, :], Mt[rows, :])
    if negate_mean:
        nc.vector.tensor_add(Qt[rows, :], Qt[rows, :], Rt[rows, :])
    else:
        nc.vector.tensor_sub(Qt[rows, :], Qt[rows, :], Rt[rows, :])
    nc.scalar.activation(Qt[rows, :], Qt[rows, :], AF.Ln,
                         scale=1.0 / n, bias=eps_c[rows, :])
    nc.scalar.activation(Rt[rows, :], Qt[rows, :], AF.Exp, scale=-0.5)


# ======================= host side =======================
_CACHE = {}


def _to_bf16(a):
    return np.ascontiguousarray(a.astype(ml_dtypes.bfloat16))


def _pack(layout_cols, ncol, P, dtype, skip=()):
    A = np.zeros((128, ncol), np.float32)
    for n, (r0, nr, c0, nc_) in layout_cols.items():
        if n in skip:
            continue
        v = P[n]
        assert v.shape == (nr, nc_), (n, v.shape, nr, nc_)
        A[r0:r0 + nr, c0:c0 + nc_] = v
    return _to_bf16(A) if dtype == "b" else A


def _host_prep(inp):
    g = {k: np.ascontiguousarray(np.asarray(v, np.float32)) for k, v in inp.items()}
    P = {}
    qbr = np.zeros((1, 128), np.float32)
    qbr[0, 32:64] = 0.25
    qbr[0, 96:128] = 0.25
    P["Bcat"] = np.concatenate([np.concatenate(
        [g["B_q"], g["B_q"], g["B_v"], g["B_v"]], 1), qbr], 0)
    Wqv = np.zeros((128, 128), np.float32)
    Wqv[0:64, 0:64] = -np.concatenate([g["Wqe"][:32], g["Wqe"][32:]], 0)
    Wqv[64:128, 64:128] = -np.concatenate([g["Wve"][:32], g["Wve"][32:]], 0)
    P["Wqv"] = Wqv
    bqp = (g["bqe"] @ g["Wq"] + g["bq"])[None, :]
    P["bqc"] = np.ascontiguousarray(bqp.reshape(4, 128).T)  # 0.125 is in kv_s
    P["vW1b"] = g["vW1"]
    P["vb1p"] = (g["bve"] @ g["vW1"] + g["vb1"])[:, None]
    vW2p = g["vg"][:, None] * g["vW2"]
    vb2p = g["vbn"] @ g["vW2"] + g["vb2"]
    P["WgamF"] = vW2p[:, :HH]
    Wbeta, bbeta = vW2p[:, HH:], vb2p[HH:]
    P["bcol"] = None  # filled below
    bgam1 = np.ascontiguousarray((1.0 + vb2p[:HH]).reshape(4, 128).T)
    P["mW1"] = g["mW1"]
    P["WbmF"] = Wbeta @ g["mW1"]
    mb1pp = np.ascontiguousarray(
        (bbeta @ g["mW1"] + g["mb1"]).reshape(4, 128).T)
    mW2p = g["mg"][:, None] * g["mW2"]
    mb2p = g["mbn"] @ g["mW2"] + g["mb2"]
    P["mW2F"] = mW2p
    P["csmW2b"] = mW2p.sum(0)[None, :]
    P["WoF"] = g["Wo"]
    P["boppF"] = (mb2p @ g["Wo"] + g["bo"])[None, :]
    P["Wq"] = g["Wq"]
    P["WkF"], P["WvF"] = g["Wk"], g["Wv"]
    P["bkv"] = np.concatenate([g["bk"][None, :], g["bv"][None, :]], 1)
    P["bcol"] = np.concatenate([bgam1, mb1pp], 1)
    for wn in ("mW1", "mW2F", "WoF"):
        P[wn] = np.ascontiguousarray(
            P[wn].reshape(4, 128, HH).transpose(1, 0, 2).reshape(128, 4 * HH))
    P["onec"] = np.ones((128, 1), np.float32)
    P["oner"] = np.ones((1, CZ), np.float32)
    selS = np.zeros((128, NCHUNK, NCHUNK), np.float32)
    for i in range(NCHUNK):
        selS[:, i, i] = 1.0
    P["selS"] = np.ascontiguousarray(selS.reshape(128, NCHUNK * NCHUNK))
    P["selF"] = P["selS"]
    mS = np.zeros((128, 4, NH), np.float32)
    for tt in range(4):
        for p in range(128):
            mS[p, tt, 2 * tt + p // 64] = 1.0
    P["maskS"] = np.ascontiguousarray(mS.reshape(128, 32))
    P["maskB"] = np.zeros((NH, HH), np.float32)
    for h in range(NH):
        P["maskB"][h, h * H:(h + 1) * H] = 1.0
    mT = np.zeros((NH, 4, 128), np.float32)
    for tt in range(4):
        for p in range(128):
            mT[2 * tt + p // 64, tt, p] = 1.0
    P["maskT"] = np.ascontiguousarray(mT.reshape(NH, 4 * 128))
    return P, g


def make_in_maps(P, g):
    base_a = _pack(CPA_COLS, CPA_NCOL, P, "f", skip=("xp",))
    base_b = _pack(CPB_COLS, CPB_NCOL, P, "f", skip=("aT",))
    ww = _pack(WPW_COLS, WPW_NCOL, P, "b")
    cd = _pack(CPD_COLS, CPD_NCOL, P, "f")
    xT_full = np.ascontiguousarray(g["inputs"].reshape(B * C, D).T)
    in_maps = []
    for core in range(NCORE):
        b = core // (NCORE // B)
        A = base_a.copy()
        r0, nr, c0, ncol = CPA_COLS["xp"]
        xrow = np.concatenate(
            [xT_full[:, core * CPC:(core + 1) * CPC], g["p"][b].T], 1)
        ones0 = np.concatenate([np.ones((1, CPC), np.float32),
                                np.zeros((1, Z), np.float32)], 1)
        A[r0:r0 + nr, c0:c0 + ncol] = np.concatenate([xrow, ones0], 0)
        Bm = base_b.copy()
        r0, nr, c0, ncol = CPB_COLS["aT"]
        Bm[r0:r0 + nr, c0:c0 + ncol] = g["a"][b].T
        in_maps.append({"cpk_a": A, "cpk_b": Bm, "wpk_w": ww, "cpk_d": cd})
    return in_maps


def kernel(**inputs):
    P, g = _host_prep(inputs)
    if "nc" not in _CACHE:
        _CACHE["nc"] = build_kernel()
    nc = _CACHE["nc"]
    in_maps = make_in_maps(P, g)
    res = run_bass_kernel_spmd(nc, in_maps, core_ids=list(range(NCORE)))
    outs = [res.results[i]["out"] for i in range(NCORE)]
    return np.concatenate(outs, 0).reshape(B, C, HH).astype(np.float32)


if __name__ == "__main__":
    import reference
    inp = {k: np.asarray(v) for k, v in reference.setup_inputs().items()}
    got = kernel(**inp)
    exp = np.asarray(reference.reference(**reference.setup_inputs()))
    err = np.abs(got - exp)
    scale = float(np.sqrt((exp ** 2).mean()))
    print("max abs err:", err.max(), " scaled:", err.max() / scale)
